# revision 1
# baseline (speedup 1.0000x reference)
"""AutoCorrelation (Autoformer) Trainium2 kernel, 8-core data-parallel over batch.

Algorithm per core (one batch b):
  mean_value[b, tau] = (1/(H*E)) sum_c circular-crosscorr(q[:,c], k[:,c])[tau]
computed via a 16-subsequence DFT-128 decomposition (t = 16u + r):
  - stage A: per (r, c): DFT_128 of subsequence -> packed spectrum (re 0..64 | im 1..63)
    as fp32 matmuls with the data as the stationary operand (output partitions = c).
  - stage P: cross-spectra P[a,b,f] = sum_c Qf[c,a,f] * conj-parts(Kf[c,b,f]) as
    tiny [K=128,M=16,N=16] matmuls accumulated over 4 c-chunks.
  - diagonal sums of the 16x16 blocks (partition-fold with shifted APs),
    twiddle combine, then one IDFT-128 matmul -> mean_value[b] (exact fp32).
  - AllReduce(sum) over the 8 cores -> scores; on-device top-7 (vector.max/max_index),
    softmax over the 7 gathered mean_value entries -> weights.
  - output: out[t,c] = sum_i w_i * v[(t+delta_i) % L, c] via transposed V in SBUF
    (PE transposes) + 7 dynamically-offset (register ds) fused multiply-adds on DVE,
    then PE transpose back.
"""

import os
import sys
import numpy as np

for p in ("/opt/trn_rl_repo",):
    if p not in sys.path and os.path.isdir(p):
        sys.path.insert(0, p)

import concourse.bass as bass
import concourse.bacc as bacc
import concourse.tile as tile
import concourse.mybir as mybir
from concourse import bass_utils
import concourse.bass_isa as bass_isa

F32 = mybir.dt.float32
F16 = mybir.dt.float16
U32 = mybir.dt.uint32
AL = mybir.AluOpType

B, L, H, E = 8, 2048, 8, 64
C = H * E            # 512
U, R = 128, 16       # L = R * U ; t = 16*u + r
NF = 65              # rfft freqs of DFT-128 we keep (0..64)
NCORES = 8
TOPK = 7


def _consts():
    u = np.arange(U)
    # stage-A DFT matrix, packed: cols 0..64 = cos, cols 65..127 = -sin (f=1..63)
    wdft = np.zeros((U, 128), dtype=np.float16)
    f = np.arange(NF)
    wdft[:, :NF] = np.cos(2 * np.pi * np.outer(u, f) / U)
    f2 = np.arange(1, 64)
    wdft[:, NF:] = -np.sin(2 * np.pi * np.outer(u, f2) / U)

    # IDFT matrix on packed spectrum -> mean_value (incl 2x Hermitian weight and 1/(U*C))
    widft = np.zeros((128, U), dtype=np.float32)
    v = np.arange(U)
    scale = np.ones(NF); scale[1:64] = 2.0
    norm = 1.0 / (U * C)
    widft[:NF, :] = (scale[:, None] * np.cos(2 * np.pi * np.outer(f, v) / U)) * norm
    widft[NF:, :] = (-2.0 * np.sin(2 * np.pi * np.outer(f2, v) / U)) * norm

    # twiddles for the lo-diagonal terms, folded into extra IDFT slabs:
    # mean_value = Widft^T @ HI + (diag(tw1) Widft)^T @ LO1 + (diag(tw2) Widft)^T @ LO2
    twv1 = np.zeros((128, 1), dtype=np.float32)
    twv2 = np.zeros((128, 1), dtype=np.float32)
    twv1[:NF, 0] = np.cos(2 * np.pi * f / U)
    twv2[:NF, 0] = -np.sin(2 * np.pi * f / U)
    twv1[NF:, 0] = np.cos(2 * np.pi * f2 / U)
    twv2[NF:, 0] = np.sin(2 * np.pi * f2 / U)
    widft_l1 = (twv1 * widft).astype(np.float32)
    widft_l2 = (twv2 * widft).astype(np.float32)

    ident = np.eye(128, dtype=np.float32)
    return wdft, widft, widft_l1, widft_l2, ident


def build_kernel(nc, no_collective=False):
    q_ext = nc.dram_tensor("q", [L, C], F16, kind="ExternalInput")
    k_ext = nc.dram_tensor("k", [L, C], F16, kind="ExternalInput")
    v_ext = nc.dram_tensor("v", [L, C], F32, kind="ExternalInput")
    wdft_ext = nc.dram_tensor("wdft", [U, 128], F16, kind="ExternalInput")
    widft_ext = nc.dram_tensor("widft", [128, U], F32, kind="ExternalInput")
    widftl1_ext = nc.dram_tensor("widftl1", [128, U], F32, kind="ExternalInput")
    widftl2_ext = nc.dram_tensor("widftl2", [128, U], F32, kind="ExternalInput")
    ident_ext = nc.dram_tensor("ident", [128, 128], F32, kind="ExternalInput")
    out_ext = nc.dram_tensor("out", [L, C], F32, kind="ExternalOutput")

    NCH = C // 128  # 4 channel chunks

    with tile.TileContext(nc) as tc:
        with (
            tc.tile_pool(name="const", bufs=1) as constp,
            tc.tile_pool(name="spec", bufs=1) as specp,
            tc.tile_pool(name="stage", bufs=2) as stagep,
            tc.tile_pool(name="vt", bufs=1) as vtp,
            tc.tile_pool(name="small", bufs=1) as smallp,
            tc.tile_pool(name="ot", bufs=3) as otp,
            tc.tile_pool(name="psA", bufs=2, space="PSUM") as psA,
            tc.tile_pool(name="psP", bufs=2, space="PSUM") as psP,
            tc.tile_pool(name="psT", bufs=3, space="PSUM") as psT,
            tc.tile_pool(name="dram", bufs=1, space="DRAM") as dramp,
        ):
            # ---- constants ----
            wdft_sb = constp.tile([U, 128], F16, tag="wdft")
            widft_sb = constp.tile([128, U], F32, tag="widft")
            widftl1_sb = constp.tile([128, U], F32, tag="widftl1")
            widftl2_sb = constp.tile([128, U], F32, tag="widftl2")
            ident_sb = constp.tile([128, 128], F32, tag="ident")
            nc.sync.dma_start(wdft_sb[:], wdft_ext.ap())
            nc.sync.dma_start(widft_sb[:], widft_ext.ap())
            nc.sync.dma_start(widftl1_sb[:], widftl1_ext.ap())
            nc.sync.dma_start(widftl2_sb[:], widftl2_ext.ap())
            nc.sync.dma_start(ident_sb[:], ident_ext.ap())

            # ---- stage A: subsequence spectra ----
            # spectra tiles: QS[ch] [c=128, r*128 + fpack]
            QS = [specp.tile([128, R * 128], F32, tag=f"qs{ch}", name=f"qs{ch}") for ch in range(NCH)]
            KS = [specp.tile([128, R * 128], F32, tag=f"ks{ch}", name=f"ks{ch}") for ch in range(NCH)]

            for side, (ext, SS) in enumerate(((q_ext, QS), (k_ext, KS))):
                # view [L, C] as [ch, u, (r c)]
                src = ext.ap().rearrange(
                    "(u r) (g c) -> g u r c", r=R, c=128
                )
                for ch in range(NCH):
                    x = stagep.tile([128, R * 128], F16, tag="xstage")
                    nc.sync.dma_start(x[:], src[ch])
                    for r4 in range(R // 4):
                        ps = psA.tile([128, 512], F32, tag="psa")
                        for rr in range(4):
                            r = r4 * 4 + rr
                            # K side: reversed r-order inside the PSUM quad so
                            # one contiguous DMA lands the reversed blocks.
                            slot = rr if side == 0 else (3 - rr)
                            nc.tensor.matmul(
                                ps[:, slot * 128:(slot + 1) * 128],
                                x[:, r * 128:(r + 1) * 128],
                                wdft_sb[:],
                                start=True, stop=True,
                            )
                        base = r4 * 4 if side == 0 else (R - 4 - r4 * 4)
                        if r4 % 4 == 3:
                            nc.scalar.activation(
                                SS[ch][:, base * 128:(base + 4) * 128], ps[:],
                                mybir.ActivationFunctionType.Copy)
                        else:
                            nc.vector.tensor_copy(
                                SS[ch][:, base * 128:(base + 4) * 128], ps[:])

            # ---- stage P: cross spectra, 16x16 per f ----
            # PP planes [16, f*16 + b]
            PPre = specp.tile([16, NF * 16], F32, tag="ppre")
            PPim = specp.tile([16, NF * 16], F32, tag="ppim")

            QSr = [QS[ch].rearrange("c (r fp) -> c fp r", fp=128) for ch in range(NCH)]
            KSr = [KS[ch].rearrange("c (r fp) -> c fp r", fp=128) for ch in range(NCH)]

            HLO = smallp.tile([128, 3 * R], F32, tag="hlo")
            LO1, HI, LO2 = HLO[:, 0:16], HLO[:, 16:32], HLO[:, 32:48]
            dgrow = dramp.tile([2 * NF * 48], F32, tag="dgrow", name="dgrow")
            rrv = dgrow[0:NF * 48].rearrange("(f x) -> f x", x=48)
            irv = dgrow[NF * 48:].rearrange("(f x) -> f x", x=48)

            # skew destinations in DRAM (affine skew: addr 3121*a + 48*f + 1 + a + j)
            skd_re = dramp.tile([16 * 3121], F32, tag="skdre", name="skd_re")
            skd_im = dramp.tile([16 * 3121], F32, tag="skdim", name="skd_im")
            DG2 = specp.tile([16, 2 * NF * 48], F32, tag="dg2")
            DGre = DG2[:, 0:NF * 48]
            DGim = DG2[:, NF * 48:]
            nc.vector.memset(DG2[:, 0:3121], 0.0)
            for SKD in (skd_re, skd_im):
                nc.sync.dma_start(
                    SKD.rearrange("(a y) -> a y", y=3121), DG2[:, 0:3121])

            fgroups = [list(range(g * 8, min(g * 8 + 8, NF))) for g in range((NF + 7) // 8)]
            for fg in fgroups:
                pg = psP.tile([16, 8 * 64], F32, tag="psp")
                for fi, f in enumerate(fg):
                    off = fi * 64
                    prods = [(0, f, f)]
                    if 0 < f < 64:
                        prods += [(16, f, 64 + f), (32, 64 + f, f),
                                  (48, 64 + f, 64 + f)]
                    # product-major: close each 4-chunk PSUM accumulation
                    # group before opening the next one in the same bank.
                    for po, qf, kf in prods:
                        for ch in range(NCH):
                            nc.tensor.matmul(
                                pg[:, off + po:off + po + 16],
                                QSr[ch][:, qf, :], KSr[ch][:, kf, :],
                                start=(ch == 0), stop=(ch == NCH - 1))
                # combine: P_re = A + D ; P_im = C' - B
                # (DVE has a single PSUM read port: stage to SBUF first)
                pgs = stagep.tile([16, 8 * 64], F32, tag="pgs", name="pgs")
                for fi, f in enumerate(fg):
                    n = 16 if (f == 0 or f == 64) else 64
                    nc.any.tensor_copy(
                        pgs[:, fi * 64:fi * 64 + n], pg[:, fi * 64:fi * 64 + n])
                pgv = pgs.rearrange("p (f x) -> p f x", x=64)
                for fi, f in enumerate(fg):
                    dst = slice(f * 16, f * 16 + 16)
                    if 0 < f < 64:
                        nc.vector.tensor_add(
                            PPre[:, dst], pgv[:, fi, 0:16], pgv[:, fi, 48:64])
                        nc.vector.tensor_sub(
                            PPim[:, dst], pgv[:, fi, 32:48], pgv[:, fi, 16:32])
                    else:
                        nc.any.tensor_copy(PPre[:, dst], pgv[:, fi, 0:16])
                        nc.vector.memset(PPim[:, dst], 0.0)
                # stream this f-range through skew -> readback -> reduce while
                # the PE keeps running the next group's matmuls
                f0, nf = fg[0], len(fg)
                for PP, SKD, DG in ((PPre, skd_re, DGre), (PPim, skd_im, DGim)):
                    skew_dst = bass.AP(
                        SKD.tensor, 1 + 48 * f0, [[3122, 16], [48, nf], [1, 16]])
                    nc.sync.dma_start(
                        skew_dst,
                        PP.rearrange("a (f j) -> a f j", j=16)[:, f0:f0 + nf, :])
                    rd_src = bass.AP(
                        SKD.tensor, 48 * f0, [[3121, 16], [1, 48 * nf]])
                    nc.sync.dma_start(DG[:, 48 * f0:48 * (f0 + nf)], rd_src)
                    nc.gpsimd.partition_all_reduce(
                        DG[:, 48 * f0:48 * (f0 + nf)],
                        DG[:, 48 * f0:48 * (f0 + nf)],
                        channels=16, reduce_op=bass_isa.ReduceOp.add)


            # ---- V transpose into [c, t] doubled ----
            BF16 = mybir.dt.bfloat16
            VT = [vtp.tile([128, 2 * L], BF16, tag=f"vt{ch}", name=f"vt{ch}") for ch in range(NCH)]
            vsrc = v_ext.ap().rearrange("(j p) (g c) -> g p j c", p=128, c=128)
            ident_bfe = constp.tile([128, 128], BF16, tag="identbfe")
            nc.vector.tensor_copy(ident_bfe[:], ident_sb[:])
            for ch in range(NCH):
                vstage = stagep.tile([128, 2048], F32, tag="vstage", name="vstage", bufs=1)
                # one 1 MB DMA per chunk: [p, (j, c)]
                nc.sync.dma_start(
                    vstage.rearrange("p (j c) -> p j c", c=128), vsrc[ch])
                vstb = stagep.tile([128, 2048], BF16, tag="vstb", name="vstb")
                nc.vector.tensor_copy(vstb[:], vstage[:])
                for j in range(L // 128):
                    pst = psT.tile([128, 128], BF16, tag="pst")
                    nc.tensor.transpose(
                        pst[:], vstb[:, j * 128:(j + 1) * 128], ident_bfe[:])
                    nc.vector.tensor_copy(VT[ch][:, j * 128:(j + 1) * 128], pst[:])
                nc.vector.tensor_copy(VT[ch][:, L:2 * L], VT[ch][:, 0:L])



            # batch scatter of the diagonal sums into HLO partitions
            nc.sync.dma_start(dgrow.rearrange("(o x) -> o x", o=1), DG2[0:1, :])
            nc.sync.dma_start(HLO[0:NF, 0:32], rrv[0:NF, 0:32])
            nc.sync.dma_start(HLO[NF:128, 0:32], irv[1:64, 0:32])
            nc.sync.dma_start(LO2[0:NF, :], irv[0:NF, 0:16])
            nc.sync.dma_start(LO2[NF:128, :], rrv[1:64, 0:16])

            ps_mv = psA.tile([128, R], F32, tag="psmv", bufs=1)
            nc.tensor.matmul(ps_mv[:], widft_sb[:], HI[:], start=True, stop=False)
            nc.tensor.matmul(ps_mv[:], widftl1_sb[:], LO1[:], start=False, stop=False)
            nc.tensor.matmul(ps_mv[:], widftl2_sb[:], LO2[:], start=False, stop=True)
            mv_sb = smallp.tile([128, R], F32, tag="mv")
            nc.any.tensor_copy(mv_sb[:], ps_mv[:])

            # ---- all-reduce scores over batch ----
            mv_dram = dramp.tile([L], F32, tag="mvd")
            sc_dram = dramp.tile([L], F32, tag="scd")
            nc.gpsimd.dma_start(mv_dram.rearrange("(p w) -> p w", w=R), mv_sb[:])
            if no_collective:
                nc.gpsimd.dma_start(sc_dram[:], mv_dram[:])
            else:
                nc.gpsimd.collective_compute(
                    "AllReduce",
                    AL.add,
                    replica_groups=[list(range(NCORES))],
                    ins=[mv_dram.opt()],
                    outs=[sc_dram.opt()],
                )

            # ---- top-7 + softmax weights ----
            sc_sb = smallp.tile([1, L], F32, tag="scsb")
            mvl_sb = smallp.tile([1, L], F32, tag="mvl")
            nc.gpsimd.dma_start(sc_sb[:], sc_dram.rearrange("(o l) -> o l", o=1))
            nc.gpsimd.dma_start(mvl_sb[:], mv_dram.rearrange("(o l) -> o l", o=1))
            mx8 = smallp.tile([1, 8], F32, tag="mx8")
            idx8 = smallp.tile([1, 8], U32, tag="idx8")
            nc.vector.max(mx8[:], sc_sb[:])
            nc.vector.max_index(idx8[:], mx8[:], sc_sb[:])

            _, deltas = nc.values_load_multi_w_load_instructions(
                idx8[0:1, 0:TOPK], min_val=0, max_val=L - 1,
                skip_runtime_bounds_check=True,
                engines=(mybir.EngineType.PE,
                         mybir.EngineType.DVE,
                         mybir.EngineType.Activation))

            wv = smallp.tile([1, 8], F32, tag="wv")
            nc.vector.memset(wv[:], 0.0)
            for i in range(TOPK):
                nc.any.tensor_copy(
                    wv[0:1, i:i + 1], mvl_sb[0:1, bass.ds(deltas[i], 1)])
            nc.scalar.activation(
                wv[0:1, 0:TOPK], wv[0:1, 0:TOPK], mybir.ActivationFunctionType.Exp)
            wsum = smallp.tile([1, 1], F32, tag="wsum")
            nc.vector.reduce_sum(wsum[:], wv[0:1, 0:TOPK], axis=mybir.AxisListType.X)
            wrec = smallp.tile([1, 1], F32, tag="wrec")
            nc.vector.reciprocal(wrec[:], wsum[:])
            nc.vector.tensor_scalar(
                wv[0:1, 0:TOPK], wv[0:1, 0:TOPK], wrec[:], None, AL.mult)
            wb = smallp.tile([128, 8], F32, tag="wb")
            nc.gpsimd.partition_broadcast(wb[:, 0:8], wv[0:1, 0:8])

            # ---- 7-tap weighted shifted sum on DVE ----
            # reference: rolled[l] = v[(l - delta) % L] -> doubled-V offset L - delta
            offs = [L - d for d in deltas]
            # taps on the PE: out_psum[c, t] += (w_i I)^T @ VT[:, off_i + slice]
            # with PSUM accumulating the 7 taps; scaled identities as lhsT.
            ident_bf2 = constp.tile([128, 128], BF16, tag="identbf2")
            nc.vector.tensor_copy(ident_bf2[:], ident_sb[:])
            WIall = constp.tile([128, TOPK * 128], BF16, tag="wiall")
            for i in range(TOPK):
                nc.vector.tensor_scalar(
                    WIall[:, i * 128:(i + 1) * 128], ident_bf2[:],
                    wb[:, i:i + 1], None, AL.mult)
            WI = [WIall[:, i * 128:(i + 1) * 128] for i in range(TOPK)]
            ACC = [specp.tile([128, L], BF16, tag=f"qs{ch}", name=f"acc{ch}") for ch in range(NCH)]
            NSUB = 4  # 512-wide sub-tiles; ks-major so stores overlap taps
            for ks in range(NSUB):
                for ch in range(NCH):
                    pt = psA.tile([128, 512], F32, tag="psa")
                    for i in range(TOPK):
                        nc.tensor.matmul(
                            pt[:], WI[i][:],
                            VT[ch][:, bass.ds(offs[i] + ks * 512, 512)],
                            start=(i == 0), stop=(i == TOPK - 1))
                    nc.vector.tensor_copy(
                        ACC[ch][:, ks * 512:(ks + 1) * 512], pt[:])


            ident_bf = constp.tile([128, 128], mybir.dt.bfloat16, tag="identbf")
            nc.vector.tensor_copy(ident_bf[:], ident_sb[:])

            # ---- transpose back and store ----
            for j in range(L // 128):
                ot = otp.tile([128, C], F32, tag="ot")
                for ch in range(NCH):
                    pst = psT.tile([128, 128], mybir.dt.bfloat16, tag="pst")
                    nc.tensor.transpose(
                        pst[:], ACC[ch][:, j * 128:(j + 1) * 128], ident_bf[:])
                    nc.vector.tensor_copy(ot[:, ch * 128:(ch + 1) * 128], pst[:])
                nc.sync.dma_start(
                    out_ext.ap().rearrange("(j p) c -> j p c", p=128)[j], ot[:])

    return nc


_NC_CACHE = {}


def _get_nc():
    if "nc" not in _NC_CACHE:
        nc = bacc.Bacc(
            "TRN2", target_bir_lowering=False, debug=False, num_devices=NCORES)
        build_kernel(nc)
        nc.compile()
        _NC_CACHE["nc"] = nc
    return _NC_CACHE["nc"]


def _in_maps(queries, keys, values):
    wdft, widft, wl1, wl2, ident = _consts()
    maps = []
    for b in range(B):
        maps.append({
            "q": np.ascontiguousarray(queries[b].reshape(L, C), dtype=np.float16),
            "k": np.ascontiguousarray(keys[b].reshape(L, C), dtype=np.float16),
            "v": np.ascontiguousarray(values[b].reshape(L, C), dtype=np.float32),
            "wdft": wdft, "widft": widft, "widftl1": wl1, "widftl2": wl2,
            "ident": ident,
        })
    return maps


def run(queries, keys, values, trace=False):
    nc = _get_nc()
    res = bass_utils.run_bass_kernel_spmd(
        nc, _in_maps(queries, keys, values),
        core_ids=list(range(NCORES)), trace=trace)
    outs = [res.results[b]["out"].reshape(L, H, E) for b in range(B)]
    return np.stack(outs, axis=0), res


def kernel(queries, keys, values, attn_mask=None):
    out, _ = run(np.asarray(queries), np.asarray(keys), np.asarray(values))
    return out.astype(np.float32)

